# revision 1
# baseline (speedup 1.0000x reference)
"""DiagPooling (segment-reduce over square-image diagonals) on 8 NeuronCores.

Input  x: [8, 128, 512, 512] f32. Output: [8, 1, 513] f32 — per batch, the
mean over (channels, diagonal) of each diagonal offset in [-256, 256].

Sharding: batch b -> core b (data parallel, no communication).

Per-core pipeline:
1. Stream the 128 channels in their natural contiguous layout ([128, 2048]
   tiles, 8 KB per partition per DMA descriptor — the fastest possible HBM
   pattern) and accumulate on VectorE: y = sum_c x[b, c]. The channel sum is
   layout-agnostic, so the expensive 128 MiB stream needs no rearrangement.
2. Re-lay the reduced y (1 MiB) via a DRAM round-trip into the stride-513
   view: P[q, r] = y_flat[513*q + r]. Since flat (i, j) = 513*i + (j - i),
   every diagonal of y is a COLUMN of P: column r holds diagonal o = r
   (rows q <= 511 - r) and o = r - 513 (rows q >= 512 - r).
3. The wanted diagonals o in [-256, 256] are selected by a precomputed mask
   folded with 1/(C*diag_len); masked column sums (per-group elementwise
   multiplies and ones-vector matmuls whose 4-group fold rides the PSUM
   accumulation) give the means.
"""

import numpy as np

import concourse.bass as bass
import concourse.bacc as bacc
import concourse.mybir as mybir
from concourse import tile
from concourse.bass_utils import run_bass_kernel_spmd

B, C, H = 8, 128, 512
R = H + 1               # 513: columns of the strided view
NG = 4                  # 512 q-rows -> 4 groups of 128 partitions
F = NG * R              # 2052: SBUF free width of the strided view
CH_ELEMS = H * H        # elements per (b, c) image
FW = CH_ELEMS // 128    # 2048: flat free width per partition
N_IN = C * CH_ELEMS
Y_PAD = CH_ELEMS + H    # 262656 = 512*513: padded scratch for the P view
F32 = mybir.dt.float32


def _mask_qr() -> np.ndarray:
    """[512, 513] f64: wanted(q, r) / (C * diag_len)."""
    q = np.arange(H, dtype=np.int64)[:, None]
    r = np.arange(R, dtype=np.int64)[None, :]
    prefix = (r <= H // 2) & (q + r <= H - 1)            # diagonal o = r
    suffix = (r > H // 2) & (q + r >= H) & (q <= H - 2)  # o = r - 513
    mask = prefix | suffix
    o = np.where(r <= H // 2, r, r - R)
    denom = float(C) * (H - np.abs(o)).astype(np.float64)
    return mask.astype(np.float64) / denom


def _build_weights() -> np.ndarray:
    """[128, F] f32: the mask in the SBUF strided-view layout
    (row q = g*128 + p -> partition p, free column g*513 + r)."""
    wqr = _mask_qr()
    return (
        wqr.reshape(NG, 128, R).transpose(1, 0, 2).reshape(128, F).astype(np.float32)
    )


def _build_program():
    nc = bacc.Bacc("TRN2", target_bir_lowering=False, debug=False, num_devices=B)
    xp = nc.dram_tensor("x", [N_IN], F32, kind="ExternalInput")
    wt = nc.dram_tensor("w", [128, F], F32, kind="ExternalInput")
    out_t = nc.dram_tensor("out", [1, R], F32, kind="ExternalOutput")
    y_dram = nc.dram_tensor("y_scratch", [Y_PAD], F32)

    NBUFS = 12

    with tile.TileContext(nc) as tc:
        with (
            tc.tile_pool(name="consts", bufs=1) as consts,
            tc.tile_pool(name="accp", bufs=1) as accp,
            tc.tile_pool(name="loadp", bufs=NBUFS) as loadp,
            tc.tile_pool(name="outp", bufs=1) as outp,
            tc.tile_pool(name="psum", bufs=2, space=bass.MemorySpace.PSUM) as psump,
        ):
            # epilogue constants load up front on the otherwise-idle scalar
            # ring / GpSimd so nothing in the tail waits for them, and the
            # sync-ring channel stream is not delayed either
            w_tile = consts.tile([128, F], F32)
            nc.scalar.dma_start(out=w_tile[:], in_=wt.ap())
            ones = consts.tile([128, 1], F32)
            nc.gpsimd.memset(ones[:], 1.0)
            # zero the scratch tail so the strided re-read never sees junk
            zpad = consts.tile([1, H], F32)
            nc.gpsimd.memset(zpad[:], 0.0)
            nc.scalar.dma_start(
                out=bass.AP(y_dram, CH_ELEMS, [[1, H]]), in_=zpad[:]
            )

            # 1. contiguous channel stream, flat accumulate (8 KB contiguous
            # per partition per DMA — the accumulate is layout-blind)
            acc = accp.tile([128, FW], F32)
            for c in range(C):
                t = loadp.tile([128, FW], F32)
                nc.sync.dma_start(
                    out=t[:], in_=bass.AP(xp, c * CH_ELEMS, [[FW, 128], [1, FW]])
                )
                if c == 0:
                    nc.vector.tensor_copy(out=acc[:], in_=t[:])
                else:
                    nc.vector.tensor_add(out=acc[:], in0=acc[:], in1=t[:])

            # 2. re-layout y through DRAM into the stride-513 view, one
            # 128-row group per DMA so masking can start as groups land.
            # The write goes in 4 partition slices: slice k covers the flat
            # range group k's re-read needs, so the tail chains pipeline.
            for p0, p1 in ((0, 33), (33, 65), (65, 97), (97, 128)):
                nc.sync.dma_start(
                    out=bass.AP(
                        y_dram, p0 * FW, [[FW, p1 - p0], [1, FW]]
                    ),
                    in_=acc[p0:p1, :],
                )
            p_tile = outp.tile([128, F], F32)
            for g in range(NG):
                nc.scalar.dma_start(
                    out=p_tile[:, g * R : (g + 1) * R],
                    in_=bass.AP(y_dram, R * 128 * g, [[R, 128], [1, R]]),
                )

            # 3. masked column sums. Per-group mask-muls pipeline with the
            # P-read arrivals; the 4-group fold uses 3 DVE adds so only TWO
            # fp32 matmuls (1/4-rate moving operand) sit in the serial tail.
            u = outp.tile([128, R], F32)
            for g in range(NG):
                sl = slice(g * R, (g + 1) * R)
                nc.vector.tensor_mul(
                    out=p_tile[:, sl], in0=p_tile[:, sl], in1=w_tile[:, sl]
                )
                if g == 1:
                    nc.vector.tensor_add(
                        out=u[:], in0=p_tile[:, 0:R], in1=p_tile[:, R : 2 * R]
                    )
                elif g > 1:
                    nc.vector.tensor_add(out=u[:], in0=u[:], in1=p_tile[:, sl])
            ps_a = psump.tile([1, 512], F32)
            ps_b = psump.tile([1, 1], F32)
            nc.tensor.matmul(ps_a[:], ones[:], u[:, 0:512], start=True, stop=True)
            nc.tensor.matmul(ps_b[:], ones[:], u[:, 512:513], start=True, stop=True)
            res = outp.tile([1, R], F32)
            nc.vector.tensor_copy(out=res[:, 0:512], in_=ps_a[:])
            nc.vector.tensor_copy(out=res[:, 512:513], in_=ps_b[:])
            nc.sync.dma_start(out=out_t.ap(), in_=res[:])

    nc.compile()
    return nc


_CACHE = {}


def kernel(x, _trace=False, _trace_cores=None) -> np.ndarray:
    x = np.asarray(x, dtype=np.float32)
    assert x.shape == (B, C, H, H), x.shape

    if "nc" not in _CACHE:
        _CACHE["nc"] = _build_program()
        _CACHE["w"] = _build_weights()
    nc = _CACHE["nc"]
    w = _CACHE["w"]

    in_maps = [
        {"x": np.ascontiguousarray(x[b]).reshape(-1), "w": w} for b in range(B)
    ]
    result = run_bass_kernel_spmd(
        nc,
        in_maps,
        core_ids=list(range(B)),
        trace=_trace,
        trace_cores=_trace_cores,
    )
    _CACHE["last_result"] = result

    out = np.empty((B, 1, R), dtype=np.float32)
    for b in range(B):
        r = result.results[b]["out"].reshape(R)
        # column r -> offset o = r (r <= 256) / r - 513 (r >= 257);
        # output index n = o + 256
        out[b, 0, :] = np.concatenate([r[R - 256 :], r[: R - 256]])
    return out

